# revision 6
# baseline (speedup 1.0000x reference)
"""Weighted 2D cross-entropy (BCE-over-classes) loss on 8 Trainium2 cores.

Math (matches the reference):
  t in [0,19); pos = t>0, neg = t==0 (all pixels are pos or neg; mask == 1)
  S(i) = sum_c bce(i,c) = -[ A(i) + B(i) ]
     A(i)   = sum_c log(1-p_c(i))
     B(i)   = log(p_t(i)) - log(1-p_t(i))
  loss = ( (NEG/TOT)*S_pos_sum + (POS/TOT)*S_neg_sum ) / (TOT*C)

Per-core (core k <- batch element k, pure data parallel), pixel grid
[128, 4096] split in two halves of [128, 2048] so half 0's tail overlaps
half 1's DMA stream:
  - per class: one 1MB DMA, ACT pass L_c = Ln(1-p_c) f32->bf16 with
    accum_out -> per-(class,half) sum (U_all contribution)
  - DVE: eq_c = (T==c) at 16-bit rate, then copy_predicated builds
    L_sel = L_{t(i)}(i) in SBUF -- each pixel written exactly once, no
    multiply and no select matmuls
  - PE identity-matmuls accumulate only A = sum_c L_c into PSUM (f32)
  - per-half tail: pos*A via STT (PSUM read), expL = exp(L_sel),
    logp = Ln(1-expL), B = logp - L_sel (bf16), masked/total sums via
    accum_out / tensor_reduce into a [128, 48] stats tile
Host folds the per-partition stats in float64 (the "all-reduce").
"""

from contextlib import ExitStack

import numpy as np

import concourse.bass as bass
import concourse.mybir as mybir
import concourse.tile as tile
from concourse import bacc
from concourse.bass_utils import run_bass_kernel_spmd

# problem shape (hardcoded per harness contract)
N, C, H, W = 8, 19, 512, 1024
PIX = H * W          # 524288 pixels per core
P = 128              # partitions
FCOLS = PIX // P     # 4096 free columns when pixels laid out [128, 4096]
HALF = FCOLS // 2    # 2048-wide halves
NHALF = 2
N_CORES = 8

DT = mybir.dt

# stats column layout (all f32)
COL_U = 0                    # 38 cols: sum L_c per (class, half) -> U_all
COL_POSA = COL_U + C * NHALF  # 2 cols: sum pos*A per half
COL_POSB = COL_POSA + NHALF   # 2 cols: sum pos*B per half
COL_SUMB = COL_POSB + NHALF   # 2 cols: sum B per half
COL_CNT = COL_SUMB + NHALF    # 2 cols: pos count per half
NCOLS = 48                    # padded


def build_kernel() -> bass.Bass:
    # Bacc (not raw Bass): its compile() pipeline runs
    # generate_event_semaphores, which splits multi-sem waits to satisfy the
    # 1-wait-per-instruction TRN2 sync structs -- raw Bass modules with
    # Tile-emitted multi-waits fail walrus codegen.
    nc = bacc.Bacc("TRN2")

    predict = nc.declare_dram_parameter("predict", [C, PIX], DT.float32, isOutput=False)
    target = nc.declare_dram_parameter("target", [P, FCOLS], DT.int32, isOutput=False)
    idn = nc.declare_dram_parameter("idn", [P, P], DT.bfloat16, isOutput=False)
    out = nc.declare_dram_parameter("out", [P, NCOLS], DT.float32, isOutput=True)

    pred_r = predict.rearrange("c (p f) -> c p f", p=P)  # [19, 128, 4096]

    with tile.TileContext(nc) as tc, ExitStack() as ctx:
        const = ctx.enter_context(tc.tile_pool(name="const", bufs=1))
        # p bufs=8 aligns slot reuse with the global DMA->DMAHW-proc
        # round-robin (8 procs), so the WAW on the old writer is same-proc
        # FIFO order and Tile emits no cross-queue wait
        p_pool = ctx.enter_context(tc.tile_pool(name="p", bufs=8))
        lm_pool = ctx.enter_context(tc.tile_pool(name="lm", bufs=5))
        eq_pool = ctx.enter_context(tc.tile_pool(name="eq", bufs=3))
        ti_pool = ctx.enter_context(tc.tile_pool(name="ti", bufs=2))
        lsel_pool = ctx.enter_context(tc.tile_pool(name="lsel", bufs=2))
        tail_pool = ctx.enter_context(tc.tile_pool(name="tail", bufs=2))
        psum_pool = ctx.enter_context(tc.tile_pool(name="ps", bufs=2, space="PSUM"))

        idn_sb = const.tile([P, P], DT.bfloat16, tag="idn")
        nc.sync.dma_start(out=idn_sb[:], in_=idn[:])

        stats = const.tile([P, NCOLS], DT.float32, tag="stats")
        nc.vector.memset(stats[:], 0.0)

        # bias=-1.0 has no pre-registered const AP; build one
        neg1 = const.tile([P, 1], DT.float32, tag="neg1")
        nc.vector.memset(neg1[:], -1.0)

        cnt_scr = const.tile([P, HALF], DT.bfloat16, tag="cntscr")

        for h in range(NHALF):
            hsl = slice(h * HALF, (h + 1) * HALF)

            t_i32 = ti_pool.tile([P, HALF], DT.int32, tag="ti")
            nc.sync.dma_start(out=t_i32[:], in_=target[:, hsl])
            t_bf = const.tile([P, HALF], DT.bfloat16, tag=f"tb{h}")
            nc.vector.tensor_copy(out=t_bf[:], in_=t_i32[:])

            # pos count up-front (also settles the DVE self-dep on t_bf so
            # later ops carry at most one sem wait)
            nc.vector.tensor_scalar(
                out=cnt_scr[:],
                in0=t_bf[:],
                scalar1=0.5,
                scalar2=None,
                op0=mybir.AluOpType.is_gt,
                op1=mybir.AluOpType.add,
                accum_out=stats[:, COL_CNT + h : COL_CNT + h + 1],
            )

            # L_sel accumulator for this half (each pixel written exactly
            # once by the copy_predicated of its target class)
            lsel = lsel_pool.tile([P, HALF], DT.bfloat16, tag="lsel")

            # PSUM accumulator: A = sum_c L_c (4 banks per half)
            acc_ps = psum_pool.tile([P, HALF], DT.float32, tag="acc")

            for c in range(C):
                p_t = p_pool.tile([P, HALF], DT.float32, tag="p")
                nc.sync.dma_start(out=p_t[:], in_=pred_r[c, :, hsl])

                # L_c = Ln(1-p) in bf16; accum_out -> sum over the half
                lm = lm_pool.tile([P, HALF], DT.bfloat16, tag="lm")
                uc = COL_U + c * NHALF + h
                nc.scalar.activation(
                    out=lm[:],
                    in_=p_t[:],
                    func=mybir.ActivationFunctionType.Ln,
                    bias=1.0,
                    scale=-1.0,
                    accum_out=stats[:, uc : uc + 1],
                )

                # eq at DVE 16-bit rate (uint16: CopyPredicated wants an int
                # mask); predicated copy gathers the target-class L into lsel
                # without a multiply
                eq = eq_pool.tile([P, HALF], DT.uint16, tag="eq")
                nc.vector.tensor_scalar(
                    out=eq[:],
                    in0=t_bf[:],
                    scalar1=float(c),
                    scalar2=None,
                    op0=mybir.AluOpType.is_equal,
                )
                nc.vector.copy_predicated(out=lsel[:], mask=eq[:], data=lm[:])

                for s in range(HALF // 512):
                    ssl = slice(s * 512, (s + 1) * 512)
                    nc.tensor.matmul(
                        acc_ps[:, ssl],
                        lhsT=idn_sb[:],
                        rhs=lm[:, ssl],
                        start=(c == 0),
                        stop=(c == C - 1),
                    )

            # ---- per-half tail ----
            # sum pos*A (PSUM f32 read; independent of the exp/ln chain)
            scr = tail_pool.tile([P, HALF], DT.float32, tag="scr")
            nc.vector.scalar_tensor_tensor(
                out=scr[:],
                in0=t_bf[:],
                scalar=0.5,
                in1=acc_ps[:],
                op0=mybir.AluOpType.is_gt,
                op1=mybir.AluOpType.mult,
                accum_out=stats[:, COL_POSA + h : COL_POSA + h + 1],
            )

            # B = log(p_t) - L_sel = Ln(exp(-L_sel) - 1), two chained ACTs:
            # expn = exp(-L_sel) = 1/(1-p_t), then B = Ln(expn - 1)
            expn = tail_pool.tile([P, HALF], DT.float32, tag="expn")
            nc.scalar.activation(
                out=expn[:],
                in_=lsel[:],
                func=mybir.ActivationFunctionType.Exp,
                scale=-1.0,
            )
            b_t = tail_pool.tile([P, HALF], DT.bfloat16, tag="b")
            nc.scalar.activation(
                out=b_t[:],
                in_=expn[:],
                func=mybir.ActivationFunctionType.Ln,
                bias=neg1[:],
            )
            scrb = tail_pool.tile([P, HALF], DT.bfloat16, tag="scrb")
            nc.vector.scalar_tensor_tensor(
                out=scrb[:],
                in0=t_bf[:],
                scalar=0.5,
                in1=b_t[:],
                op0=mybir.AluOpType.is_gt,
                op1=mybir.AluOpType.mult,
                accum_out=stats[:, COL_POSB + h : COL_POSB + h + 1],
            )
            nc.vector.tensor_reduce(
                out=stats[:, COL_SUMB + h : COL_SUMB + h + 1],
                in_=b_t[:],
                axis=mybir.AxisListType.X,
                op=mybir.AluOpType.add,
            )

        nc.sync.dma_start(out=out[:], in_=stats[:])

    if not nc.is_finalized():
        nc.finalize()

    return nc


def combine(outs) -> np.float32:
    """Fold the 8 cores' [128, 48] stats tiles into the scalar loss."""
    tot = np.float64(0.0)
    s_all = np.float64(0.0)
    s_pos = np.float64(0.0)
    pos = np.float64(0.0)
    for st in outs:
        st = st.astype(np.float64)
        u_all = st[:, COL_U : COL_U + C * NHALF].sum()
        pos_a = st[:, COL_POSA : COL_POSA + NHALF].sum()
        pos_b = st[:, COL_POSB : COL_POSB + NHALF].sum()
        sum_b = st[:, COL_SUMB : COL_SUMB + NHALF].sum()
        cnt = st[:, COL_CNT : COL_CNT + NHALF].sum()
        s_all += -(sum_b + u_all)
        s_pos += -(pos_b + pos_a)
        pos += cnt
        tot += PIX
    neg = tot - pos
    s_neg = s_all - s_pos
    loss = ((neg / tot) * s_pos + (pos / tot) * s_neg) / (tot * C)
    return np.float32(loss)


_NC_CACHE = None


def kernel(predict: np.ndarray, target: np.ndarray) -> np.ndarray:
    global _NC_CACHE
    if _NC_CACHE is None:
        _NC_CACHE = build_kernel()
    nc = _NC_CACHE

    import ml_dtypes

    predict = np.ascontiguousarray(predict, dtype=np.float32)
    target = np.ascontiguousarray(target, dtype=np.int32)
    idn = np.eye(P, dtype=np.float32).astype(ml_dtypes.bfloat16)

    in_maps = []
    for k in range(N_CORES):
        in_maps.append(
            {
                "predict": predict[k].reshape(C, PIX),
                "target": target[k].reshape(P, FCOLS),
                "idn": idn,
            }
        )

    res = run_bass_kernel_spmd(nc, in_maps, list(range(N_CORES)))
    return combine([res.results[k]["out"] for k in range(N_CORES)])
